# revision 32
# baseline (speedup 1.0000x reference)
"""Trainium2 Bass kernel for 16-head self-attention (B=2, S=2048, D=1024).

Sharding: 8 cores = 2 batches x 4 head-groups (4 heads each).  Wq/Wk/Wv are
column-split, Wo row-split (tensor parallel over heads) + data parallel over
batch.  Each core computes a partial [S, D] output; host sums the 4 partials
per batch (the TP reduce) and stacks the 2 batches.

Host-side prep (inside kernel(), off the device clock): x and mask are
pre-transposed and cast to bf16, weights pre-transposed/cast, so the device
does zero layout work -- every PE cycle is a real matmul.

Fused, software-pipelined per-core schedule.  The attention inner loop is
paced by the ScalarE exp stream (~2.1us per 128-key chunk vs ~1.7us of PE
matmul), so all other PE work -- next q-block projection and previous
q-block output projection -- is drip-fed into those gaps ("pending" closure
queue, drained two slots per key chunk):
  1. Lead-in: kT = (k @ Wk.T).T d-major; v = [x @ Wv.T | 1] s-major with a
     ones column per head (65 cols per head); q-block 0.
  2. Per q-block j: for each key chunk: scoresT[k,q] = kT-slice.T @ qT (two
     heads packed via PE row tiling), exp via ACT (1/8 scale folded) -> bf16,
     multiply by pre-transposed bf16 mask tile (free-dim broadcast over the
     head pair), then per head ctx[65,q] += [v|1].T @ attnT -- row 64 of each
     ctx PSUM tile accumulates the softmax denominator for free.  Softmax
     max-subtraction is skipped: scores ~ N(0,1) so fp32 exp is safe; masked
     entries are exactly zeroed by the multiply.
  3. Normalize (eager, at block end): 4 reciprocal rows -> two K=2
     pattern-matmul partition-broadcasts (one per head pair) -> multiply into
     ctxT on PSUM eviction.
  4. Output projection rows for block j (ctxT.T @ woT) and the q-projection
     for block j+2 are enqueued as pending closures, drained inside the next
     attention block.
  All PSUM flows through one rotating 2x2-bank pool (scores / projections /
  broadcasts share the "sc" tag) plus 4 single-bank ctx accumulators:
  exactly 8 banks.
"""

import sys
from contextlib import ExitStack

import numpy as np

sys.path.insert(0, "/opt/trn_rl_repo")

import concourse.bacc as bacc
import concourse.mybir as mybir
import concourse.tile as tile
from concourse.bass import ds, ts

B, S, D, H = 2, 2048, 1024, 16
DK = D // H  # 64
NCORES = 8
GH = H // (NCORES // B)  # 4 heads per core
GD = GH * DK  # 256 projected dims per core

F32 = mybir.dt.float32
BF16 = mybir.dt.bfloat16

P = 128
NQ = 512  # q free-dim chunk in the attention loop


def build_nc(s=S, d=D, gh=GH, dk=DK):
    gd = gh * dk
    SC = s // P  # key chunks of 128
    DC = d // P
    GDC = gd // P
    JC = s // NQ
    HPAIRS = gh // 2
    NQP = NQ // P

    nc = bacc.Bacc("TRN2", target_bir_lowering=False)
    xqT = nc.dram_tensor("xqT", [d, s], BF16, kind="ExternalInput")
    xkT = nc.dram_tensor("xkT", [d, s], BF16, kind="ExternalInput")
    xvT = nc.dram_tensor("xvT", [d, s], BF16, kind="ExternalInput")
    maskT = nc.dram_tensor("maskT", [s, s], BF16, kind="ExternalInput")
    wq = nc.dram_tensor("wq", [d, gd], BF16, kind="ExternalInput")
    wk = nc.dram_tensor("wk", [d, gd], BF16, kind="ExternalInput")
    wv = nc.dram_tensor("wv", [d, gd], BF16, kind="ExternalInput")
    wo = nc.dram_tensor("wo", [gd, d], BF16, kind="ExternalInput")
    out = nc.dram_tensor("out", [s, d], BF16, kind="ExternalOutput")

    scale = float(1.0 / np.sqrt(dk))
    CTX_DELAY = 9  # head-pair steps the attn@V accumulation trails the exp stream

    with tile.TileContext(nc) as tc, ExitStack() as top:
        consts = top.enter_context(tc.tile_pool(name="consts", bufs=1))
        qkv = top.enter_context(tc.tile_pool(name="qkv", bufs=1))
        wpool = top.enter_context(tc.tile_pool(name="wpool", bufs=1))
        xtp = top.enter_context(tc.tile_pool(name="xtp", bufs=3))
        mstage = top.enter_context(tc.tile_pool(name="mstage", bufs=6))
        attnp = top.enter_context(tc.tile_pool(name="attnp", bufs=CTX_DELAY + 4))
        smalls = top.enter_context(tc.tile_pool(name="smalls", bufs=2))
        outp = top.enter_context(tc.tile_pool(name="outp", bufs=3))
        ps_att = top.enter_context(tc.tile_pool(name="ps_att", bufs=2, space="PSUM"))
        ps_ctx = top.enter_context(tc.tile_pool(name="ps_ctx", bufs=1, space="PSUM"))

        qT = qkv.tile([P, GDC, s], BF16, tag="qT")  # [gd, s] d-major
        kT = qkv.tile([P, GDC, s], BF16, tag="kT")
        vb = qkv.tile([P, SC, gh, dk + 1], BF16, tag="v")  # [s, gd] + ones col
        nc.any.memset(vb[:, :, :, dk : dk + 1], 1.0)
        ctxT = qkv.tile([P, GDC, s], BF16, tag="ctxT")  # [gd, s] d-major

        # wk via the hardware DGE queue ahead of everything: the first
        # projection matmul only needs wk + the first k-block
        wk_t = wpool.tile([P, DC, gd], BF16, tag="wk")
        nc.sync.dma_start(wk_t[:], wk.rearrange("(c p) m -> p c m", p=P))

        def load_x(x_dram, jb):
            xt = xtp.tile([P, DC, NQ], BF16, tag="xt", name=f"xt_{x_dram.name}{jb}")
            nc.sync.dma_start(
                xt[:],
                x_dram.rearrange("(c p) m -> p c m", p=P)[:, :, ds(jb * NQ, NQ)],
            )
            return xt

        def proj_mms(w_t, jb, xt, tag):
            """(x @ W.T).T for one 512-col block; both output chunks'
            accumulation chains interleaved so PE never waits on PSUM."""
            pp = ps_att.tile([P, 2, NQ], F32, tag="sc", name=f"pp_{tag}{jb}")
            for kc in range(DC):
                for mc in range(GDC):
                    nc.tensor.matmul(
                        pp[:, mc],
                        w_t[:, kc, ts(mc, P)],
                        xt[:, kc],
                        start=(kc == 0),
                        stop=(kc == DC - 1),
                    )
            return pp

        def proj_v(jb, xt):
            """vb s-major block: [x @ Wv.T] into the 64-wide head slots."""
            for half in range(NQP // 2):
                pv = ps_att.tile([P, 2, NQ], F32, tag="sc", name=f"pv_{jb}_{half}")
                for kc in range(DC):
                    for sq2 in range(2):
                        sq = half * 2 + sq2
                        nc.tensor.matmul(
                            pv[:, sq2, :gd],
                            xt[:, kc, ts(sq, P)],
                            wv_t[:, kc, :],
                            start=(kc == 0),
                            stop=(kc == DC - 1),
                        )
                nc.vector.tensor_copy(
                    vb[:, ds(jb * NQP + half * 2, 2), :, 0:dk],
                    pv[:, :, :gd].rearrange("p a (h e) -> p a h e", h=gh),
                )

        def enqueue_proj_v(jb):
            """Deferred v-projection for block jb, drip-fed into attention 0.
            ctx(0, kc) only consumes vb chunk kc after the CTX_DELAY lag, so
            these drain comfortably ahead of their consumers."""
            xt_box = [None]
            pv_box = [None, None]

            pending.append(lambda: xt_box.__setitem__(0, load_x(xvT, jb)))

            def mms(half, kc0):
                def f():
                    if pv_box[half] is None:
                        pv_box[half] = ps_att.tile(
                            [P, 2, NQ], F32, tag="sc", name=f"pv_{jb}_{half}"
                        )
                    pv = pv_box[half]
                    for kc in (kc0, kc0 + 1):
                        for sq2 in range(2):
                            sq = half * 2 + sq2
                            nc.tensor.matmul(
                                pv[:, sq2, :gd],
                                xt_box[0][:, kc, ts(sq, P)],
                                wv_t[:, kc, :],
                                start=(kc == 0),
                                stop=(kc == DC - 1),
                            )
                return f

            def evict(half):
                def f():
                    nc.vector.tensor_copy(
                        vb[:, ds(jb * NQP + half * 2, 2), :, 0:dk],
                        pv_box[half][:, :, :gd].rearrange("p a (h e) -> p a h e", h=gh),
                    )
                return f

            for half in range(NQP // 2):
                for kc0 in range(0, DC, 2):
                    pending.append(mms(half, kc0))
                pending.append(evict(half))

        # ---- software pipeline: pending closures drained inside attention ----
        # `pending` holds elastic PE filler work (q-projection for the next
        # block, output projection for the previous one).  `ctx_fifo` delays
        # each head-pair's attn@V accumulation by CTX_DELAY steps so that at a
        # block boundary the normalize chain of block j overlaps the first
        # score/exp steps of block j+1 instead of stalling the in-order PE
        # queue.
        pending = []
        ctx_fifo = []  # entries: (j, closure)

        def drain(n):
            for _ in range(n):
                if pending:
                    pending.pop(0)()

        def push_ctx(j, f):
            # burst-drain anything left from the previous block first, so its
            # normalize chain starts executing immediately
            burst = 3
            while burst and ctx_fifo and ctx_fifo[0][0] < j:
                ctx_fifo.pop(0)[1]()
                burst -= 1
            ctx_fifo.append((j, f))
            while len(ctx_fifo) > CTX_DELAY:
                ctx_fifo.pop(0)[1]()

        def flush_ctx():
            while ctx_fifo:
                ctx_fifo.pop(0)[1]()

        def enqueue_qproj(jnext):
            pp_box = [None]
            xt_box = [None]

            def loader():
                xt_box[0] = load_x(xqT, jnext)

            def mms(kc0):
                def f():
                    pp = pp_box[0]
                    if pp is None:
                        pp = ps_att.tile(
                            [P, 2, NQ], F32, tag="sc", name=f"pp_q{jnext}"
                        )
                        pp_box[0] = pp
                    for kc in (kc0, kc0 + 1):
                        for mc in range(GDC):
                            nc.tensor.matmul(
                                pp[:, mc],
                                wq_t[:, kc, ts(mc, P)],
                                xt_box[0][:, kc],
                                start=(kc == 0),
                                stop=(kc == DC - 1),
                            )
                return f

            pending.append(loader)
            for kc0 in range(0, DC, 2):
                pending.append(mms(kc0))

            def evict():
                nc.vector.tensor_copy(qT[:, :, ds(jnext * NQ, NQ)], pp_box[0][:])

            pending.append(evict)

        def enqueue_ph3(j):
            def make(sq):
                po_box = [None]

                def mms(kc):
                    def f():
                        po = po_box[0]
                        if po is None:
                            po = ps_att.tile(
                                [P, 2, NQ], F32, tag="sc", name=f"po_{j}_{sq}"
                            )
                            po_box[0] = po
                        scr = j * NQP + sq
                        for nj in range(d // NQ):
                            nc.tensor.matmul(
                                po[:, nj],
                                ctxT[:, kc, ts(scr, P)],
                                wo_t[:, kc, ds(nj * NQ, NQ)],
                                start=(kc == 0),
                                stop=(kc == GDC - 1),
                            )
                    return f

                def evict_dma():
                    scr = j * NQP + sq
                    ot = outp.tile([P, d], BF16, tag="ot", name=f"ot_{j}_{sq}")
                    nc.vector.tensor_copy(
                        ot[:].rearrange("p (a q) -> p a q", a=2), po_box[0][:]
                    )
                    nc.sync.dma_start(out[ts(scr, P), :], ot[:])

                for kc in range(GDC):
                    pending.append(mms(kc))
                pending.append(evict_dma)

            for sq in range(NQP):
                make(sq)

        def attn(j):
            ctx_ps = [
                ps_ctx.tile([dk + 1, NQ], F32, tag=f"ctx{h}", name=f"ctx_ps{h}_{j}")
                for h in range(gh)
            ]
            step = 0
            for kc in range(SC):
                mT = mstage.tile([P, NQ], BF16, tag="mT", name=f"mT_{j}_{kc}")
                nc.sync.dma_start(
                    mT[:],
                    maskT.rearrange("(c p) q -> p c q", p=P)[:, kc, ds(j * NQ, NQ)],
                )
                for hp in range(HPAIRS):
                    heads = (2 * hp, 2 * hp + 1)
                    sc_ps = ps_att.tile(
                        [P, 2, NQ], F32, tag="sc", name=f"sc_{j}_{kc}_{hp}"
                    )
                    for i, h in enumerate(heads):
                        mc, off = divmod(h * dk, P)
                        nc.tensor.matmul(
                            sc_ps[:, i],
                            kT[:, mc, ts(kc, P)][ds(off, dk), :],
                            qT[:, mc, ds(j * NQ, NQ)][ds(off, dk), :],
                            start=True,
                            stop=True,
                            tile_position=(off, 0),
                        )
                    at = attnp.tile(
                        [P, 2, NQ], BF16, tag="at", name=f"at_{j}_{kc}_{hp}"
                    )
                    nc.scalar.activation(
                        at[:], sc_ps[:], mybir.ActivationFunctionType.Exp,
                        scale=scale,
                    )
                    nc.vector.tensor_tensor(
                        at[:],
                        at[:],
                        mT[:].unsqueeze(1).to_broadcast([P, 2, NQ]),
                        op=mybir.AluOpType.mult,
                    )
                    if step >= 4:
                        drain(1)  # fill the exp/mask-mult wait with filler PE work
                    if j == JC - 1 and step >= 22:
                        # last block: run the delayed-ctx queue dry so the
                        # final normalize chain starts before attention ends
                        for _ in range(2):
                            if ctx_fifo:
                                ctx_fifo.pop(0)[1]()
                    step += 1

                    def ctx_mms(at=at, kc=kc, heads=heads):
                        for i, h in enumerate(heads):
                            nc.tensor.matmul(
                                ctx_ps[h][:],
                                vb[:, kc, h, :],
                                at[:, i],
                                start=(kc == 0),
                                stop=(kc == SC - 1),
                            )

                    push_ctx(j, ctx_mms)
            return ctx_ps

        def normalize(j, ctx_ps):
            """Denominators sit in row dk of each ctx PSUM tile; recip /
            broadcast / evict-multiply interleaved per head so each ctx tile
            frees as early as possible."""
            rec = smalls.tile([1, gh, NQ], BF16, tag="rec", name=f"rec_{j}")
            bc_sb = smalls.tile([dk, gh, NQ], BF16, tag="bcs", name=f"bcs_{j}")
            # per head pair: two reciprocal rows -> one GPSIMD
            # partition-broadcast (no PE, no PSUM, so the scores ring and the
            # exp stream never couple to this chain) -> two evict-multiplies
            for hp in range(HPAIRS):
                with nc.allow_low_precision(reason="bf16 softmax denom reciprocal"):
                    for i in range(2):
                        nc.vector.reciprocal(
                            rec[:, 2 * hp + i], ctx_ps[2 * hp + i][dk : dk + 1, :]
                        )
                nc.gpsimd.partition_broadcast(
                    bc_sb[:, ds(2 * hp, 2), :], rec[:, ds(2 * hp, 2), :]
                )
            for h in range(gh):
                nc.vector.tensor_tensor(
                    ctxT[ds((h % 2) * dk, dk), h // 2, ds(j * NQ, NQ)],
                    ctx_ps[h][0:dk, :],
                    bc_sb[:, h, :],
                    op=mybir.AluOpType.mult,
                )

        # ---------------- emission ----------------
        xt_k0 = load_x(xkT, 0)
        wv_t = wpool.tile([P, DC, gd], BF16, tag="wv")
        nc.gpsimd.dma_start(wv_t[:], wv.rearrange("(c p) m -> p c m", p=P))
        wq_t = wpool.tile([P, DC, gd], BF16, tag="wq")
        nc.gpsimd.dma_start(wq_t[:], wq.rearrange("(c p) m -> p c m", p=P))

        pp = proj_mms(wk_t, 0, xt_k0, "k")
        nc.scalar.copy(kT[:, :, ds(0, NQ)], pp[:])
        for jb in range(1, JC):
            pp = proj_mms(wk_t, jb, load_x(xkT, jb), "k")
            nc.scalar.copy(kT[:, :, ds(jb * NQ, NQ)], pp[:])
        wo_t = wpool.tile([P, GDC, d], BF16, tag="wo")
        nc.gpsimd.dma_start(wo_t[:], wo.rearrange("(c p) m -> p c m", p=P))
        for jb in range(JC):
            proj_v(jb, load_x(xvT, jb))
        pp = proj_mms(wq_t, 0, load_x(xqT, 0), "q")
        nc.scalar.copy(qT[:, :, ds(0, NQ)], pp[:])

        for j in range(JC):
            if j + 1 < JC:
                enqueue_qproj(j + 1)
            ctx_ps = attn(j)

            def norm_then_ph3(j=j, ctx_ps=ctx_ps):
                normalize(j, ctx_ps)
                enqueue_ph3(j)

            push_ctx(j, norm_then_ph3)
        flush_ctx()
        drain(len(pending))

    nc.finalize()
    return nc


_NC_CACHE = {}


def get_nc(**kw):
    key = tuple(sorted(kw.items()))
    if key not in _NC_CACHE:
        _NC_CACHE[key] = build_nc(**kw)
    return _NC_CACHE[key]


def shard_inputs(q, k, v, mask, Wq, Wk, Wv, Wo):
    import ml_dtypes

    BF = ml_dtypes.bfloat16
    q = np.asarray(q, dtype=np.float32)
    k = np.asarray(k, dtype=np.float32)
    v = np.asarray(v, dtype=np.float32)
    mask = np.asarray(mask, dtype=np.int32)
    Wq, Wk, Wv, Wo = (np.asarray(w, dtype=np.float32) for w in (Wq, Wk, Wv, Wo))

    per_batch = []
    for b in range(B):
        per_batch.append(
            {
                "xqT": np.ascontiguousarray(q[b].T.astype(BF)),
                "xkT": np.ascontiguousarray(k[b].T.astype(BF)),
                "xvT": np.ascontiguousarray(v[b].T.astype(BF)),
                "maskT": np.ascontiguousarray(mask[b, 0].T.astype(BF)),
            }
        )

    in_maps = []
    for c in range(NCORES):
        b, g = divmod(c, NCORES // B)
        rows = slice(g * GD, (g + 1) * GD)
        m = dict(per_batch[b])
        m["wq"] = np.ascontiguousarray(Wq[rows, :].T.astype(BF))
        m["wk"] = np.ascontiguousarray(Wk[rows, :].T.astype(BF))
        m["wv"] = np.ascontiguousarray(Wv[rows, :].T.astype(BF))
        m["wo"] = np.ascontiguousarray(Wo[:, rows].T.astype(BF))
        in_maps.append(m)
    return in_maps


def kernel(q, k, v, mask, Wq, Wk, Wv, Wo):
    from concourse.bass_utils import run_bass_kernel_spmd

    nc = get_nc()
    in_maps = shard_inputs(q, k, v, mask, Wq, Wk, Wv, Wo)
    res = run_bass_kernel_spmd(nc, in_maps, list(range(NCORES))).results
    out = np.zeros((B, S, D), dtype=np.float32)
    for c in range(NCORES):
        out[c // (NCORES // B)] += np.asarray(res[c]["out"], dtype=np.float32)
    return out


if __name__ == "__main__":
    nc = build_nc()
    print("built ok")


# revision 33
# speedup vs baseline: 6.5174x; 6.5174x over previous
"""Trainium2 Bass kernel for 16-head self-attention (B=2, S=2048, D=1024).

Sharding: 8 cores = 2 batches x 4 head-groups (4 heads each).  Wq/Wk/Wv are
column-split, Wo row-split (tensor parallel over heads) + data parallel over
batch.  Each core computes a partial [S, D] output; host sums the 4 partials
per batch (the TP reduce) and stacks the 2 batches.

Host-side prep (inside kernel(), off the device clock): x and mask are
pre-transposed and cast to bf16, weights pre-transposed/cast, so the device
does zero layout work -- every PE cycle is a real matmul.

Fused, software-pipelined per-core schedule.  The attention inner loop is
paced by the ScalarE exp stream (~2.1us per 128-key chunk vs ~1.7us of PE
matmul), so all other PE work -- next q-block projection and previous
q-block output projection -- is drip-fed into those gaps ("pending" closure
queue, drained two slots per key chunk):
  1. Lead-in: kT = (k @ Wk.T).T d-major; v = [x @ Wv.T | 1] s-major with a
     ones column per head (65 cols per head); q-block 0.
  2. Per q-block j: for each key chunk: scoresT[k,q] = kT-slice.T @ qT (two
     heads packed via PE row tiling), exp via ACT (1/8 scale folded) -> bf16,
     multiply by pre-transposed bf16 mask tile (free-dim broadcast over the
     head pair), then per head ctx[65,q] += [v|1].T @ attnT -- row 64 of each
     ctx PSUM tile accumulates the softmax denominator for free.  Softmax
     max-subtraction is skipped: scores ~ N(0,1) so fp32 exp is safe; masked
     entries are exactly zeroed by the multiply.
  3. Normalize (eager, at block end): 4 reciprocal rows -> two K=2
     pattern-matmul partition-broadcasts (one per head pair) -> multiply into
     ctxT on PSUM eviction.
  4. Output projection rows for block j (ctxT.T @ woT) and the q-projection
     for block j+2 are enqueued as pending closures, drained inside the next
     attention block.
  All PSUM flows through one rotating 2x2-bank pool (scores / projections /
  broadcasts share the "sc" tag) plus 4 single-bank ctx accumulators:
  exactly 8 banks.
"""

import sys
from contextlib import ExitStack

import numpy as np

sys.path.insert(0, "/opt/trn_rl_repo")

import concourse.bacc as bacc
import concourse.mybir as mybir
import concourse.tile as tile
from concourse.bass import ds, ts

B, S, D, H = 2, 2048, 1024, 16
DK = D // H  # 64
NCORES = 8
GH = H // (NCORES // B)  # 4 heads per core
GD = GH * DK  # 256 projected dims per core

F32 = mybir.dt.float32
BF16 = mybir.dt.bfloat16

P = 128
NQ = 512  # q free-dim chunk in the attention loop


def build_nc(s=S, d=D, gh=GH, dk=DK):
    gd = gh * dk
    SC = s // P  # key chunks of 128
    DC = d // P
    GDC = gd // P
    JC = s // NQ
    HPAIRS = gh // 2
    NQP = NQ // P

    nc = bacc.Bacc("TRN2", target_bir_lowering=False)
    xqT = nc.dram_tensor("xqT", [d, s], BF16, kind="ExternalInput")
    xkT = nc.dram_tensor("xkT", [d, s], BF16, kind="ExternalInput")
    xvT = nc.dram_tensor("xvT", [d, s], BF16, kind="ExternalInput")
    maskT = nc.dram_tensor("maskT", [s, s], BF16, kind="ExternalInput")
    wq = nc.dram_tensor("wq", [d, gd], BF16, kind="ExternalInput")
    wk = nc.dram_tensor("wk", [d, gd], BF16, kind="ExternalInput")
    wv = nc.dram_tensor("wv", [d, gd], BF16, kind="ExternalInput")
    wo = nc.dram_tensor("wo", [gd, d], BF16, kind="ExternalInput")
    out = nc.dram_tensor("out", [s, d], BF16, kind="ExternalOutput")

    scale = float(1.0 / np.sqrt(dk))
    CTX_DELAY = 9  # head-pair steps the attn@V accumulation trails the exp stream

    with tile.TileContext(nc) as tc, ExitStack() as top:
        consts = top.enter_context(tc.tile_pool(name="consts", bufs=1))
        qkv = top.enter_context(tc.tile_pool(name="qkv", bufs=1))
        wpool = top.enter_context(tc.tile_pool(name="wpool", bufs=1))
        xtp = top.enter_context(tc.tile_pool(name="xtp", bufs=3))
        mstage = top.enter_context(tc.tile_pool(name="mstage", bufs=6))
        attnp = top.enter_context(tc.tile_pool(name="attnp", bufs=CTX_DELAY + 4))
        smalls = top.enter_context(tc.tile_pool(name="smalls", bufs=2))
        outp = top.enter_context(tc.tile_pool(name="outp", bufs=3))
        ps_att = top.enter_context(tc.tile_pool(name="ps_att", bufs=2, space="PSUM"))
        ps_ctx = top.enter_context(tc.tile_pool(name="ps_ctx", bufs=1, space="PSUM"))

        qT = qkv.tile([P, GDC, s], BF16, tag="qT")  # [gd, s] d-major
        kT = qkv.tile([P, GDC, s], BF16, tag="kT")
        vb = qkv.tile([P, SC, gh, dk + 1], BF16, tag="v")  # [s, gd] + ones col
        nc.any.memset(vb[:, :, :, dk : dk + 1], 1.0)
        ctxT = qkv.tile([P, GDC, s], BF16, tag="ctxT")  # [gd, s] d-major

        # wk via the hardware DGE queue ahead of everything: the first
        # projection matmul only needs wk + the first k-block
        wk_t = wpool.tile([P, DC, gd], BF16, tag="wk")
        nc.sync.dma_start(wk_t[:], wk.rearrange("(c p) m -> p c m", p=P))

        def load_x(x_dram, jb, eng=None):
            xt = xtp.tile([P, DC, NQ], BF16, tag="xt", name=f"xt_{x_dram.name}{jb}")
            (eng or nc.sync).dma_start(
                xt[:],
                x_dram.rearrange("(c p) m -> p c m", p=P)[:, :, ds(jb * NQ, NQ)],
            )
            return xt

        def proj_mms(w_t, jb, xt, tag):
            """(x @ W.T).T for one 512-col block; both output chunks'
            accumulation chains interleaved so PE never waits on PSUM."""
            pp = ps_att.tile([P, 2, NQ], F32, tag="sc", name=f"pp_{tag}{jb}")
            for kc in range(DC):
                for mc in range(GDC):
                    nc.tensor.matmul(
                        pp[:, mc],
                        w_t[:, kc, ts(mc, P)],
                        xt[:, kc],
                        start=(kc == 0),
                        stop=(kc == DC - 1),
                    )
            return pp

        def proj_v(jb, xt):
            """vb s-major block: [x @ Wv.T] into the 64-wide head slots."""
            for half in range(NQP // 2):
                pv = ps_att.tile([P, 2, NQ], F32, tag="sc", name=f"pv_{jb}_{half}")
                for kc in range(DC):
                    for sq2 in range(2):
                        sq = half * 2 + sq2
                        nc.tensor.matmul(
                            pv[:, sq2, :gd],
                            xt[:, kc, ts(sq, P)],
                            wv_t[:, kc, :],
                            start=(kc == 0),
                            stop=(kc == DC - 1),
                        )
                nc.vector.tensor_copy(
                    vb[:, ds(jb * NQP + half * 2, 2), :, 0:dk],
                    pv[:, :, :gd].rearrange("p a (h e) -> p a h e", h=gh),
                )

        def enqueue_proj_v(jb):
            """Deferred v-projection for block jb, drip-fed into attention 0.
            ctx(0, kc) only consumes vb chunk kc after the CTX_DELAY lag, so
            these drain comfortably ahead of their consumers."""
            xt_box = [None]
            pv_box = [None, None]

            pending.append(lambda: xt_box.__setitem__(0, load_x(xvT, jb)))

            def mms(half, kc0):
                def f():
                    if pv_box[half] is None:
                        pv_box[half] = ps_att.tile(
                            [P, 2, NQ], F32, tag="sc", name=f"pv_{jb}_{half}"
                        )
                    pv = pv_box[half]
                    for kc in (kc0, kc0 + 1):
                        for sq2 in range(2):
                            sq = half * 2 + sq2
                            nc.tensor.matmul(
                                pv[:, sq2, :gd],
                                xt_box[0][:, kc, ts(sq, P)],
                                wv_t[:, kc, :],
                                start=(kc == 0),
                                stop=(kc == DC - 1),
                            )
                return f

            def evict(half):
                def f():
                    nc.vector.tensor_copy(
                        vb[:, ds(jb * NQP + half * 2, 2), :, 0:dk],
                        pv_box[half][:, :, :gd].rearrange("p a (h e) -> p a h e", h=gh),
                    )
                return f

            for half in range(NQP // 2):
                for kc0 in range(0, DC, 2):
                    pending.append(mms(half, kc0))
                pending.append(evict(half))

        # ---- software pipeline: pending closures drained inside attention ----
        # `pending` holds elastic PE filler work (q-projection for the next
        # block, output projection for the previous one).  `ctx_fifo` delays
        # each head-pair's attn@V accumulation by CTX_DELAY steps so that at a
        # block boundary the normalize chain of block j overlaps the first
        # score/exp steps of block j+1 instead of stalling the in-order PE
        # queue.
        pending = []
        ctx_fifo = []  # entries: (j, closure)

        def drain(n):
            for _ in range(n):
                if pending:
                    pending.pop(0)()

        def push_ctx(j, f):
            # burst-drain anything left from the previous block first, so its
            # normalize chain starts executing immediately
            burst = 3
            while burst and ctx_fifo and ctx_fifo[0][0] < j:
                ctx_fifo.pop(0)[1]()
                burst -= 1
            ctx_fifo.append((j, f))
            while len(ctx_fifo) > CTX_DELAY:
                ctx_fifo.pop(0)[1]()

        def flush_ctx():
            while ctx_fifo:
                ctx_fifo.pop(0)[1]()

        def enqueue_qproj(jnext):
            pp_box = [None]
            xt_box = [None]

            def loader():
                xt_box[0] = load_x(xqT, jnext)

            def mms(kc0):
                def f():
                    pp = pp_box[0]
                    if pp is None:
                        pp = ps_att.tile(
                            [P, 2, NQ], F32, tag="sc", name=f"pp_q{jnext}"
                        )
                        pp_box[0] = pp
                    for kc in (kc0, kc0 + 1):
                        for mc in range(GDC):
                            nc.tensor.matmul(
                                pp[:, mc],
                                wq_t[:, kc, ts(mc, P)],
                                xt_box[0][:, kc],
                                start=(kc == 0),
                                stop=(kc == DC - 1),
                            )
                return f

            pending.append(loader)
            for kc0 in range(0, DC, 2):
                pending.append(mms(kc0))

            def evict():
                nc.vector.tensor_copy(qT[:, :, ds(jnext * NQ, NQ)], pp_box[0][:])

            pending.append(evict)

        def enqueue_ph3(j):
            def make(sq):
                po_box = [None]

                def mms(kc):
                    def f():
                        po = po_box[0]
                        if po is None:
                            po = ps_att.tile(
                                [P, 2, NQ], F32, tag="sc", name=f"po_{j}_{sq}"
                            )
                            po_box[0] = po
                        scr = j * NQP + sq
                        for nj in range(d // NQ):
                            nc.tensor.matmul(
                                po[:, nj],
                                ctxT[:, kc, ts(scr, P)],
                                wo_t[:, kc, ds(nj * NQ, NQ)],
                                start=(kc == 0),
                                stop=(kc == GDC - 1),
                            )
                    return f

                def evict_dma():
                    scr = j * NQP + sq
                    ot = outp.tile([P, d], BF16, tag="ot", name=f"ot_{j}_{sq}")
                    nc.vector.tensor_copy(
                        ot[:].rearrange("p (a q) -> p a q", a=2), po_box[0][:]
                    )
                    nc.sync.dma_start(out[ts(scr, P), :], ot[:])

                for kc in range(GDC):
                    pending.append(mms(kc))
                pending.append(evict_dma)

            for sq in range(NQP):
                make(sq)

        def attn(j, boundaries=None):
            ctx_ps = [
                ps_ctx.tile([dk + 1, NQ], F32, tag=f"ctx{h}", name=f"ctx_ps{h}_{j}")
                for h in range(gh)
            ]
            step = 0
            for kc in range(SC):
                mT = mstage.tile([P, NQ], BF16, tag="mT", name=f"mT_{j}_{kc}")
                nc.sync.dma_start(
                    mT[:],
                    maskT.rearrange("(c p) q -> p c q", p=P)[:, kc, ds(j * NQ, NQ)],
                )
                for hp in range(HPAIRS):
                    heads = (2 * hp, 2 * hp + 1)
                    sc_ps = ps_att.tile(
                        [P, 2, NQ], F32, tag="sc", name=f"sc_{j}_{kc}_{hp}"
                    )
                    for i, h in enumerate(heads):
                        mc, off = divmod(h * dk, P)
                        nc.tensor.matmul(
                            sc_ps[:, i],
                            kT[:, mc, ts(kc, P)][ds(off, dk), :],
                            qT[:, mc, ds(j * NQ, NQ)][ds(off, dk), :],
                            start=True,
                            stop=True,
                            tile_position=(off, 0),
                        )
                    at = attnp.tile(
                        [P, 2, NQ], BF16, tag="at", name=f"at_{j}_{kc}_{hp}"
                    )
                    nc.scalar.activation(
                        at[:], sc_ps[:], mybir.ActivationFunctionType.Exp,
                        scale=scale,
                    )
                    nc.vector.tensor_tensor(
                        at[:],
                        at[:],
                        mT[:].unsqueeze(1).to_broadcast([P, 2, NQ]),
                        op=mybir.AluOpType.mult,
                    )
                    if step >= 4:
                        drain(1)  # fill the exp/mask-mult wait with filler PE work
                    if j == JC - 1 and step >= 22:
                        # last block: run the delayed-ctx queue dry so the
                        # final normalize chain starts before attention ends
                        for _ in range(2):
                            if ctx_fifo:
                                ctx_fifo.pop(0)[1]()
                    step += 1
                if boundaries and kc in boundaries:
                    boundaries[kc]()

                    def ctx_mms(at=at, kc=kc, heads=heads):
                        for i, h in enumerate(heads):
                            nc.tensor.matmul(
                                ctx_ps[h][:],
                                vb[:, kc, h, :],
                                at[:, i],
                                start=(kc == 0),
                                stop=(kc == SC - 1),
                            )

                    push_ctx(j, ctx_mms)
            return ctx_ps

        def normalize(j, ctx_ps):
            """Denominators sit in row dk of each ctx PSUM tile; recip /
            broadcast / evict-multiply interleaved per head so each ctx tile
            frees as early as possible."""
            rec = smalls.tile([1, gh, NQ], BF16, tag="rec", name=f"rec_{j}")
            bc_sb = smalls.tile([dk, gh, NQ], BF16, tag="bcs", name=f"bcs_{j}")
            # per head pair: two reciprocal rows -> one GPSIMD
            # partition-broadcast (no PE, no PSUM, so the scores ring and the
            # exp stream never couple to this chain) -> two evict-multiplies
            for hp in range(HPAIRS):
                with nc.allow_low_precision(reason="bf16 softmax denom reciprocal"):
                    for i in range(2):
                        nc.vector.reciprocal(
                            rec[:, 2 * hp + i], ctx_ps[2 * hp + i][dk : dk + 1, :]
                        )
                nc.gpsimd.partition_broadcast(
                    bc_sb[:, ds(2 * hp, 2), :], rec[:, ds(2 * hp, 2), :]
                )
            for h in range(gh):
                nc.vector.tensor_tensor(
                    ctxT[ds((h % 2) * dk, dk), h // 2, ds(j * NQ, NQ)],
                    ctx_ps[h][0:dk, :],
                    bc_sb[:, h, :],
                    op=mybir.AluOpType.mult,
                )

        # ---------------- emission ----------------
        xt_k0 = load_x(xkT, 0)
        wv_t = wpool.tile([P, DC, gd], BF16, tag="wv")
        nc.gpsimd.dma_start(wv_t[:], wv.rearrange("(c p) m -> p c m", p=P))
        wq_t = wpool.tile([P, DC, gd], BF16, tag="wq")
        nc.gpsimd.dma_start(wq_t[:], wq.rearrange("(c p) m -> p c m", p=P))

        pp = proj_mms(wk_t, 0, xt_k0, "k")
        nc.scalar.copy(kT[:, :, ds(0, NQ)], pp[:])
        for jb in range(1, JC):
            pp = proj_mms(wk_t, jb, load_x(xkT, jb), "k")
            nc.scalar.copy(kT[:, :, ds(jb * NQ, NQ)], pp[:])
        wo_t = wpool.tile([P, GDC, d], BF16, tag="wo")
        nc.gpsimd.dma_start(wo_t[:], wo.rearrange("(c p) m -> p c m", p=P))
        for jb in range(JC):
            proj_v(jb, load_x(xvT, jb))
        pp = proj_mms(wq_t, 0, load_x(xqT, 0), "q")
        nc.scalar.copy(qT[:, :, ds(0, NQ)], pp[:])

        for j in range(JC):
            if j + 1 < JC:
                enqueue_qproj(j + 1)
            ctx_ps = attn(j)

            def norm_then_ph3(j=j, ctx_ps=ctx_ps):
                normalize(j, ctx_ps)
                enqueue_ph3(j)

            push_ctx(j, norm_then_ph3)
        flush_ctx()
        drain(len(pending))

    nc.finalize()
    return nc


_NC_CACHE = {}


def get_nc(**kw):
    key = tuple(sorted(kw.items()))
    if key not in _NC_CACHE:
        _NC_CACHE[key] = build_nc(**kw)
    return _NC_CACHE[key]


def shard_inputs(q, k, v, mask, Wq, Wk, Wv, Wo):
    import ml_dtypes

    BF = ml_dtypes.bfloat16
    q = np.asarray(q, dtype=np.float32)
    k = np.asarray(k, dtype=np.float32)
    v = np.asarray(v, dtype=np.float32)
    mask = np.asarray(mask, dtype=np.int32)
    Wq, Wk, Wv, Wo = (np.asarray(w, dtype=np.float32) for w in (Wq, Wk, Wv, Wo))

    per_batch = []
    for b in range(B):
        per_batch.append(
            {
                "xqT": np.ascontiguousarray(q[b].T.astype(BF)),
                "xkT": np.ascontiguousarray(k[b].T.astype(BF)),
                "xvT": np.ascontiguousarray(v[b].T.astype(BF)),
                "maskT": np.ascontiguousarray(mask[b, 0].T.astype(BF)),
            }
        )

    in_maps = []
    for c in range(NCORES):
        b, g = divmod(c, NCORES // B)
        rows = slice(g * GD, (g + 1) * GD)
        m = dict(per_batch[b])
        m["wq"] = np.ascontiguousarray(Wq[rows, :].T.astype(BF))
        m["wk"] = np.ascontiguousarray(Wk[rows, :].T.astype(BF))
        m["wv"] = np.ascontiguousarray(Wv[rows, :].T.astype(BF))
        m["wo"] = np.ascontiguousarray(Wo[:, rows].T.astype(BF))
        in_maps.append(m)
    return in_maps


def kernel(q, k, v, mask, Wq, Wk, Wv, Wo):
    from concourse.bass_utils import run_bass_kernel_spmd

    nc = get_nc()
    in_maps = shard_inputs(q, k, v, mask, Wq, Wk, Wv, Wo)
    res = run_bass_kernel_spmd(nc, in_maps, list(range(NCORES))).results
    out = np.zeros((B, S, D), dtype=np.float32)
    for c in range(NCORES):
        out[c // (NCORES // B)] += np.asarray(res[c]["out"], dtype=np.float32)
    return out


if __name__ == "__main__":
    nc = build_nc()
    print("built ok")
